# revision 1
# baseline (speedup 1.0000x reference)
"""Trainium2 Bass kernel for nn_CoherentLoss (histogram_binning).

Math: the coherent-state overlap gt[n] depends on trajectory n only through its
phase-space bin (qb, pb).  With bin centers qc, pc:

  G(qb,pb) = norm * e^{i*pc*qc} * sum_m [wpsi_m * e^{-(x_m-qc)^2}] * e^{-i*pc*x_m}

The m-sum is a matmul  V[qb,m] @ [cos|sin](pc_pb * x_m)[m, pb].  The m-axis
(2401 grid points, padded to 3072 = 8 cores x 3 tiles x 128) is sharded across
the 8 NeuronCores; each core emits a partial [Q, 256] = [Fc | Fs] slab and the
host sums the 8 slabs (a 64KB reduction) and assembles the O(N) tail: binning
indices, compact-bin scatter-add, and the final sum of squares.

Device structure per core (m within a tile is affine in the partition index p:
x[p,t] = a_t + h*p, so index-structured matrices are low-rank in p x (t,bin)
and are generated on the TensorEngine from tiny coefficient rows):

  ph[p, (t,j)] = pc_j*x[p,t]/2pi          K=2 matmul  [1,p] x coeffs
  dsq[p, (t,q)] = (qc_q - x[p,t])^2       K=3 matmul  [1,p,p^2] x coeffs
  ee = Exp(-dsq)               one ACT instruction (one table load)
  red = ph - rne(ph)           VE f32->i32->f32 round-trip (range reduction)
  cs = Sin(2pi*red)            one ACT instruction; [cos|sin] via +1/4 turn
  vv = wpsi * ee               VE per-tile scalar mul
  F += vv_t^T @ cs_t           K=128 accumulating matmuls -> PSUM [Q,256]

ACT Sin is only accurate on [-pi, pi], hence the explicit range reduction.
The tail clears semaphores (no barrier) so the NEFF is re-runnable under
profiling.
"""
from contextlib import ExitStack

import numpy as np

import concourse.bass as bass
from concourse import mybir
from concourse.bass_utils import run_bass_kernel_spmd

QMIN, QMAX, QBINS = -8.0, 8.0, 128
PMIN, PMAX, PBINS = -10.0, 10.0, 128
GAMMA = 1.0
NORM = float((2.0 * GAMMA / np.pi) ** 0.25)
TWO_PI = float(2.0 * np.pi)

N_CORES = 8
f32 = np.float32

_BUILD_CACHE = {}


def _build(T, Q):
    """Build the SPMD program: T m-tiles of 128 per core, Q occupied qb rows."""
    nc = bass.Bass()
    dt = mybir.dt.float32
    CS = 2 * PBINS            # 256: [cos | sin] per tile
    WCS = T * CS              # 768
    WQ = T * Q                # 192
    WR = WCS + WQ             # 960: rhs coefficient columns
    KK = 3 + 2 * T            # lhsT rows: ones, p, sgn x T, p^2, lnw x T

    coef_in = nc.declare_dram_parameter("coef", [KK, 128 + WR], dt, isOutput=False)
    out = nc.declare_dram_parameter("out", [Q, CS], dt, isOutput=True)

    with ExitStack() as ctx:
        coef = ctx.enter_context(nc.sbuf_tensor("coef_s", [KK, 128 + WR], dt))
        ki = ctx.enter_context(nc.sbuf_tensor("ki", [128, WCS], mybir.dt.int32))
        kf = ctx.enter_context(nc.sbuf_tensor("kf", [128, WCS], dt))
        red = ctx.enter_context(nc.sbuf_tensor("red", [128, WCS], dt))
        cs = ctx.enter_context(nc.sbuf_tensor("cs", [128, WCS], mybir.dt.bfloat16))
        vv = ctx.enter_context(nc.sbuf_tensor("vv", [128, WQ], mybir.dt.bfloat16))
        outs = ctx.enter_context(nc.sbuf_tensor("outs", [Q, CS], dt))
        scr = ctx.enter_context(nc.sbuf_tensor("scr", [1, 8], dt))
        ph = ctx.enter_context(nc.psum_tensor("ph", [128, WCS], dt))
        dq = ctx.enter_context(nc.psum_tensor("dq", [128, WQ], dt))
        ps = ctx.enter_context(nc.psum_tensor("ps", [Q, CS], dt))
        d1 = ctx.enter_context(nc.semaphore("d1"))
        p1 = ctx.enter_context(nc.semaphore("p1"))
        v1 = ctx.enter_context(nc.semaphore("v1"))
        a1 = ctx.enter_context(nc.semaphore("a1"))
        m1 = ctx.enter_context(nc.semaphore("m1"))
        c1 = ctx.enter_context(nc.semaphore("c1"))
        g1 = ctx.enter_context(nc.semaphore("g1"))
        blk = nc.Block(no_gpsimd_drain=True)
        block = blk.__enter__()

        @block.sync
        def _(sync):
            sync.dma_start(out=coef[:, :], in_=coef_in[:, :]).then_inc(d1, 16)

        @block.tensor
        def _(tensor):
            tensor.wait_ge(d1, 16)
            # envelope exponent arg first (all KK rows; sgn rows zero-coefficiented)
            # so ACT's Exp runs early and the Sin table load hides afterwards
            tensor.matmul(dq[:, :], lhsT=coef[:, 0:128], rhs=coef[:, 128 + WCS:128 + WR],
                          start=True, stop=True).then_inc(p1, 1)
            # phase/2pi: rows [ones, p, sgn x T], split 512 + 256 (PSUM bank limit)
            tensor.matmul(ph[:, 0:512], lhsT=coef[0:2 + T, 0:128], rhs=coef[0:2 + T, 128:128 + 512],
                          start=True, stop=True).then_inc(p1, 1)
            tensor.matmul(ph[:, 512:WCS], lhsT=coef[0:2 + T, 0:128], rhs=coef[0:2 + T, 128 + 512:128 + WCS],
                          start=True, stop=True).then_inc(p1, 1)
            # accumulating contraction over m, pipelined against the split Sin
            tensor.wait_ge(a1, 3)   # vv + cs cols [0:512] (tiles 0, 1)
            tensor.matmul(ps[:, :], lhsT=vv[:, 0:Q], rhs=cs[:, 0:CS],
                          start=True, stop=False)
            tensor.matmul(ps[:, :], lhsT=vv[:, Q:2 * Q], rhs=cs[:, CS:2 * CS],
                          start=False, stop=False)
            tensor.wait_ge(a1, 4)   # cs cols [512:768] (tile 2)
            tensor.matmul(ps[:, :], lhsT=vv[:, 2 * Q:3 * Q], rhs=cs[:, 2 * CS:3 * CS],
                          start=False, stop=True).then_inc(m1, 1)

        @block.vector
        def _(vector):
            vector.wait_ge(p1, 2)
            vector.tensor_copy(ki[:, 0:512], ph[:, 0:512])      # f32 -> i32 (rne)
            vector.tensor_copy(kf[:, 0:512], ki[:, 0:512])      # i32 -> f32
            vector.tensor_sub(red[:, 0:512], ph[:, 0:512], kf[:, 0:512]).then_inc(v1, 1)
            vector.wait_ge(p1, 3)
            vector.tensor_copy(ki[:, 512:WCS], ph[:, 512:WCS])
            vector.tensor_copy(kf[:, 512:WCS], ki[:, 512:WCS])
            vector.tensor_sub(red[:, 512:WCS], ph[:, 512:WCS], kf[:, 512:WCS]).then_inc(v1, 1)


        @block.scalar
        def _(scalar):
            # warm the Exp table while DMAs are in flight
            scalar.activation(scr[:, :], scr[:, :], mybir.ActivationFunctionType.Exp).then_inc(a1, 1)
            scalar.wait_ge(p1, 1)
            scalar.activation(vv[:, :], dq[:, :], mybir.ActivationFunctionType.Exp,
                              scale=-1.0).then_inc(a1, 1)
            # warm the Sin table while the VE range reduction runs
            scalar.activation(scr[:, :], scr[:, :], mybir.ActivationFunctionType.Sin)
            scalar.wait_ge(v1, 1)
            scalar.activation(cs[:, 0:512], red[:, 0:512], mybir.ActivationFunctionType.Sin,
                              scale=TWO_PI).then_inc(a1, 1)
            scalar.wait_ge(v1, 2)
            scalar.activation(cs[:, 512:WCS], red[:, 512:WCS], mybir.ActivationFunctionType.Sin,
                              scale=TWO_PI).then_inc(a1, 1)
            scalar.wait_ge(m1, 1)
            scalar.copy(outs[:, :], ps[:, :])
            scalar.dma_start(out=out[:, :], in_=outs[:, :]).then_inc(g1, 16)


        @block.gpsimd
        def _(gpsimd):
            gpsimd.wait_ge(g1, 16)
            # restore sem/DMA state for re-execution without a full barrier:
            # every other engine's last effect was already awaited on this chain
            lo = min(s.num for s in (d1, p1, v1, a1, m1, c1, g1))
            hi = max(s.num for s in (d1, p1, v1, a1, m1, c1, g1))
            gpsimd.dma_reset(range(lo, hi + 1))
            gpsimd.sem_clear(range(lo, hi + 1))

        # manual block exit: branch every engine to end_bb + per-engine drain,
        # but skip the all-engine event-semaphore barrier (costs ~2.5us; the
        # g1-gated sem_clear already guarantees a clean re-executable state)
        for engine, last_body in block.last_body.items():
            with nc.body(last_body, parent=nc.cur_bb, allow_existing_parent=True):
                engine.br(block.end_bb)
        nc.switch_bb(block.end_bb)
        gpsimd_type = nc.gpsimd.engine
        for eng_type, eng in nc.engines.items():
            if eng_type == gpsimd_type:
                continue
            dr = mybir.InstDrain(
                name=nc.get_next_instruction_name(), ins=[], outs=[],
                bass_is_fusable=False,
            )
            dr.engine = eng_type
            eng.add_instruction(dr)

    return nc


def _host_prep(q_re, q_im, p_re, p_im, x, psi):
    qf = q_re - p_im / f32(2.0)
    pf = f32(2.0) * q_im + p_re
    dq = f32((QMAX - QMIN) / QBINS)
    dp = f32((PMAX - PMIN) / PBINS)
    qb = np.floor((qf - f32(QMIN)) / dq)
    pb = np.floor((pf - f32(PMIN)) / dp)
    bins = (qb * PBINS + pb).astype(np.int32).reshape(-1)
    uniq, inv = np.unique(bins, return_inverse=True)
    qbi = qb.astype(np.int64).reshape(-1)
    pbi = pb.astype(np.int64).reshape(-1)
    qb_occ = np.unique(qbi)
    qb_rank = {v: i for i, v in enumerate(qb_occ)}
    qb_row = np.array([qb_rank[v] for v in qbi], dtype=np.int64)
    qc_occ = (qb_occ.astype(f32) + f32(0.5)) * dq + f32(QMIN)
    pc_all = (np.arange(PBINS, dtype=f32) + f32(0.5)) * dp + f32(PMIN)
    dx = np.diff(x)
    w = np.zeros_like(x)
    w[0] = dx[0] / 2
    w[-1] = dx[-1] / 2
    w[1:-1] = (dx[:-1] + dx[1:]) / 2
    wpsi = (w * psi).astype(f32)
    return bins, uniq, inv, qb_row, pbi, qc_occ, pc_all, wpsi


def _run_device(x, wpsi, qc_occ, pc_all, trace=False):
    M = x.shape[0]
    Qocc = qc_occ.shape[0]
    Q = max(8, int(np.ceil(Qocc / 8.0)) * 8)
    assert Q <= 128, "qb occupancy exceeds one PSUM partition tile"
    T = int(np.ceil(M / (N_CORES * 128.0)))
    Mp = N_CORES * T * 128
    CS = 2 * PBINS
    KK = 3 + 2 * T

    # grid is uniform: x[m] = x0 + m*h
    h = f32((float(x[-1]) - float(x[0])) / (M - 1))
    x0 = f32(x[0])

    wp = np.zeros(Mp, dtype=f32)
    wp[:M] = wpsi
    qc_pad = np.full(Q, 1000.0, dtype=f32)   # pad columns -> V = 0
    qc_pad[:Qocc] = qc_occ

    ws = wp.reshape(N_CORES, T, 128)

    key = (T, Q)
    if key not in _BUILD_CACHE:
        _BUILD_CACHE[key] = _build(T, Q)
    nc = _BUILD_CACHE[key]

    p_idx = np.arange(128, dtype=f32)
    r1 = (pc_all * h / f32(TWO_PI)).astype(f32)

    in_maps = []
    for c in range(N_CORES):
        # x[p, t] = a_t + h*p for this core's tiles
        a_t = (x0 + h * (np.arange(T, dtype=f32) + f32(c * T)) * f32(128.0)).astype(f32)
        wct = ws[c]                                   # [T, 128]
        # lhsT rows: ones, p, sgn x T (0.5 where wpsi<0), p^2, lnw x T (clamped)
        lhs = np.zeros((KK, 128), dtype=f32)
        lhs[0] = 1.0
        lhs[1] = p_idx
        lhs[2 + T] = p_idx * p_idx
        with np.errstate(divide="ignore"):
            lnw = np.log(np.abs(wct)).astype(f32)
        lnw = np.maximum(lnw, f32(-100.0))
        for t in range(T):
            lhs[2 + t] = f32(0.5) * (wct[t] < 0)
            lhs[3 + T + t] = lnw[t]
        rhs = np.zeros((KK, T * CS + T * Q), dtype=f32)
        for t in range(T):
            base = t * CS
            r0 = (pc_all * a_t[t] / f32(TWO_PI)).astype(f32)
            rhs[0, base:base + PBINS] = r0 + f32(0.25)   # cos args (quarter turn)
            rhs[0, base + PBINS:base + CS] = r0          # sin args
            rhs[1, base:base + CS] = np.concatenate([r1, r1])
            rhs[2 + t, base:base + CS] = 1.0             # sign half-turn for tile t
            qbase = T * CS + t * Q
            dqa = (qc_pad - a_t[t]).astype(f32)
            rhs[0, qbase:qbase + Q] = dqa * dqa
            rhs[1, qbase:qbase + Q] = f32(-2.0) * h * dqa
            rhs[2 + T, qbase:qbase + Q] = h * h
            rhs[3 + T + t, qbase:qbase + Q] = -1.0       # -ln|wpsi| for tile t
        in_maps.append({"coef": np.ascontiguousarray(np.concatenate([lhs, rhs], axis=1))})

    res = run_bass_kernel_spmd(nc, in_maps, core_ids=list(range(N_CORES)), trace=trace)
    F = np.zeros((Q, CS), dtype=np.float64)
    for c in range(N_CORES):
        F += res.results[c]["out"]
    F = F.astype(f32)
    return F[:Qocc, :PBINS], F[:Qocc, PBINS:], res


def kernel(factors_re, factors_im, q_re, q_im, p_re, p_im, x, psi):
    factors_re = np.asarray(factors_re, dtype=f32)
    factors_im = np.asarray(factors_im, dtype=f32)
    q_re = np.asarray(q_re, dtype=f32)
    q_im = np.asarray(q_im, dtype=f32)
    p_re = np.asarray(p_re, dtype=f32)
    p_im = np.asarray(p_im, dtype=f32)
    x = np.asarray(x, dtype=f32)
    psi = np.asarray(psi, dtype=f32)

    bins, uniq, inv, qb_row, pbi, qc_occ, pc_all, wpsi = _host_prep(
        q_re, q_im, p_re, p_im, x, psi
    )
    Fc, Fs, _ = _run_device(x, wpsi, qc_occ, pc_all)

    # ---- host tail: phase correction, gather, scatter-add, loss ----
    phi = (qc_occ[:, None] * pc_all[None, :]).astype(f32)
    cphi = np.cos(phi, dtype=f32)
    sphi = np.sin(phi, dtype=f32)
    G_re = f32(NORM) * (cphi * Fc + sphi * Fs)
    G_im = f32(NORM) * (sphi * Fc - cphi * Fs)
    gt_re = G_re[qb_row, pbi]
    gt_im = G_im[qb_row, pbi]

    e = np.exp((q_im * q_im).astype(f32), dtype=f32)
    ang = (p_re * q_im).astype(f32)
    pr = np.clip(np.nan_to_num(f32(NORM) * e * np.cos(ang, dtype=f32)), -100.0, 100.0).astype(f32)
    pi_ = np.clip(np.nan_to_num(f32(NORM) * e * np.sin(ang, dtype=f32)), -100.0, 100.0).astype(f32)
    vr = (pr * factors_re - pi_ * factors_im).astype(f32).reshape(-1)
    vi = (pr * factors_im + pi_ * factors_re).astype(f32).reshape(-1)

    N = vr.size
    B_re = np.zeros(N, dtype=f32)
    B_im = np.zeros(N, dtype=f32)
    np.add.at(B_re, inv, vr)
    np.add.at(B_im, inv, vi)
    dr = B_re - gt_re
    di = B_im - gt_im
    loss = np.sum(dr * dr + di * di, dtype=f32)
    return np.sqrt(loss, dtype=f32)



# revision 2
# speedup vs baseline: 1.1760x; 1.1760x over previous
"""Trainium2 Bass kernel for nn_CoherentLoss (histogram_binning).

Math: the coherent-state overlap gt[n] depends on trajectory n only through its
phase-space bin (qb, pb).  With bin centers qc, pc:

  G(qb,pb) = norm * e^{i*pc*qc} * sum_m [wpsi_m * e^{-(x_m-qc)^2}] * e^{-i*pc*x_m}

The m-sum is a matmul  V[qb,m] @ [cos|sin](pc_pb * x_m)[m, pb].  The m-axis
(2401 grid points, padded to 3072 = 8 cores x 3 tiles x 128) is sharded across
the 8 NeuronCores; each core emits a partial [Q, 256] = [Fc | Fs] slab and the
host sums the 8 slabs (a 64KB reduction) and assembles the O(N) tail: binning
indices, compact-bin scatter-add, and the final sum of squares.

Device structure per core (m within a tile is affine in the partition index p:
x[p,t] = a_t + h*p, so index-structured matrices are low-rank in p x (t,bin)
and are generated on the TensorEngine from tiny coefficient rows):

  ph[p, (t,j)] = pc_j*x[p,t]/2pi          K=2 matmul  [1,p] x coeffs
  dsq[p, (t,q)] = (qc_q - x[p,t])^2       K=3 matmul  [1,p,p^2] x coeffs
  ee = Exp(-dsq)               one ACT instruction (one table load)
  red = ph - rne(ph)           VE f32->i32->f32 round-trip (range reduction)
  cs = Sin(2pi*red)            one ACT instruction; [cos|sin] via +1/4 turn
  vv = wpsi * ee               VE per-tile scalar mul
  F += vv_t^T @ cs_t           K=128 accumulating matmuls -> PSUM [Q,256]

ACT Sin is only accurate on [-pi, pi], hence the explicit range reduction.
The tail clears semaphores (no barrier) so the NEFF is re-runnable under
profiling.
"""
from contextlib import ExitStack

import numpy as np

import concourse.bass as bass
import concourse.bass_utils as _bass_utils
from concourse import mybir
from concourse.bass_utils import run_bass_kernel_spmd

# The walrus-generated NEFF epilogue clears every allocatable semaphore one
# EVENT_SEMAPHORE at a time (~8us for 254 sems).  This kernel uses <16; cap
# the allocatable range so the epilogue is proportionally short.
if not getattr(_bass_utils, "_ant_sem_cap", False):
    _bass_utils._ant_sem_cap = True
    _orig_walrus_args = _bass_utils.get_walrus_args

    def _walrus_args_capped(*args, **kwargs):
        return _orig_walrus_args(*args, **kwargs) + ["--max-sem-num=32"]

    _bass_utils.get_walrus_args = _walrus_args_capped

QMIN, QMAX, QBINS = -8.0, 8.0, 128
PMIN, PMAX, PBINS = -10.0, 10.0, 128
GAMMA = 1.0
NORM = float((2.0 * GAMMA / np.pi) ** 0.25)
TWO_PI = float(2.0 * np.pi)

N_CORES = 8
f32 = np.float32

_BUILD_CACHE = {}


def _build(T, Q):
    """Build the SPMD program: T m-tiles of 128 per core, Q occupied qb rows."""
    nc = bass.Bass()
    dt = mybir.dt.float32
    CS = 2 * PBINS            # 256: [cos | sin] per tile
    WCS = T * CS              # 768
    WQ = T * Q                # 192
    WR = WCS + WQ             # 960: rhs coefficient columns
    KK = 3 + 2 * T            # lhsT rows: ones, p, sgn x T, p^2, lnw x T

    coef_in = nc.declare_dram_parameter("coef", [KK, 128 + WR], dt, isOutput=False)
    out = nc.declare_dram_parameter("out", [Q, CS], dt, isOutput=True)

    with ExitStack() as ctx:
        coef = ctx.enter_context(nc.sbuf_tensor("coef_s", [KK, 128 + WR], dt))
        ki = ctx.enter_context(nc.sbuf_tensor("ki", [128, WCS], mybir.dt.int32))
        kf = ctx.enter_context(nc.sbuf_tensor("kf", [128, WCS], dt))
        red = ctx.enter_context(nc.sbuf_tensor("red", [128, WCS], dt))
        cs = ctx.enter_context(nc.sbuf_tensor("cs", [128, WCS], mybir.dt.bfloat16))
        vv = ctx.enter_context(nc.sbuf_tensor("vv", [128, WQ], mybir.dt.bfloat16))
        outs = ctx.enter_context(nc.sbuf_tensor("outs", [Q, CS], dt))
        scr = ctx.enter_context(nc.sbuf_tensor("scr", [1, 8], dt))
        ph = ctx.enter_context(nc.psum_tensor("ph", [128, WCS], dt))
        dq = ctx.enter_context(nc.psum_tensor("dq", [128, WQ], dt))
        ps = ctx.enter_context(nc.psum_tensor("ps", [Q, CS], dt))
        d1 = ctx.enter_context(nc.semaphore("d1"))
        p1 = ctx.enter_context(nc.semaphore("p1"))
        v1 = ctx.enter_context(nc.semaphore("v1"))
        a1 = ctx.enter_context(nc.semaphore("a1"))
        m1 = ctx.enter_context(nc.semaphore("m1"))
        c1 = ctx.enter_context(nc.semaphore("c1"))
        g1 = ctx.enter_context(nc.semaphore("g1"))
        blk = nc.Block(no_gpsimd_drain=True)
        block = blk.__enter__()

        @block.sync
        def _(sync):
            sync.dma_start(out=coef[:, :], in_=coef_in[:, :]).then_inc(d1, 16)

        @block.tensor
        def _(tensor):
            tensor.wait_ge(d1, 16)
            # envelope exponent arg first (all KK rows; sgn rows zero-coefficiented)
            # so ACT's Exp runs early and the Sin table load hides afterwards
            tensor.matmul(dq[:, :], lhsT=coef[:, 0:128], rhs=coef[:, 128 + WCS:128 + WR],
                          start=True, stop=True).then_inc(p1, 1)
            # phase/2pi: rows [ones, p, sgn x T], split 512 + 256 (PSUM bank limit)
            tensor.matmul(ph[:, 0:512], lhsT=coef[0:2 + T, 0:128], rhs=coef[0:2 + T, 128:128 + 512],
                          start=True, stop=True).then_inc(p1, 1)
            tensor.matmul(ph[:, 512:WCS], lhsT=coef[0:2 + T, 0:128], rhs=coef[0:2 + T, 128 + 512:128 + WCS],
                          start=True, stop=True).then_inc(p1, 1)
            # accumulating contraction over m, pipelined against the split Sin
            tensor.wait_ge(a1, 3)   # vv + cs cols [0:512] (tiles 0, 1)
            tensor.matmul(ps[:, :], lhsT=vv[:, 0:Q], rhs=cs[:, 0:CS],
                          start=True, stop=False)
            tensor.matmul(ps[:, :], lhsT=vv[:, Q:2 * Q], rhs=cs[:, CS:2 * CS],
                          start=False, stop=False)
            tensor.wait_ge(a1, 4)   # cs cols [512:768] (tile 2)
            tensor.matmul(ps[:, :], lhsT=vv[:, 2 * Q:3 * Q], rhs=cs[:, 2 * CS:3 * CS],
                          start=False, stop=True).then_inc(m1, 1)

        @block.vector
        def _(vector):
            vector.wait_ge(p1, 2)
            vector.tensor_copy(ki[:, 0:512], ph[:, 0:512])      # f32 -> i32 (rne)
            vector.tensor_copy(kf[:, 0:512], ki[:, 0:512])      # i32 -> f32
            vector.tensor_sub(red[:, 0:512], ph[:, 0:512], kf[:, 0:512]).then_inc(v1, 1)
            vector.wait_ge(p1, 3)
            vector.tensor_copy(ki[:, 512:WCS], ph[:, 512:WCS])
            vector.tensor_copy(kf[:, 512:WCS], ki[:, 512:WCS])
            vector.tensor_sub(red[:, 512:WCS], ph[:, 512:WCS], kf[:, 512:WCS]).then_inc(v1, 1)


        @block.scalar
        def _(scalar):
            # warm the Exp table while DMAs are in flight
            scalar.activation(scr[:, :], scr[:, :], mybir.ActivationFunctionType.Exp).then_inc(a1, 1)
            scalar.wait_ge(p1, 1)
            scalar.activation(vv[:, :], dq[:, :], mybir.ActivationFunctionType.Exp,
                              scale=-1.0).then_inc(a1, 1)
            # warm the Sin table while the VE range reduction runs
            scalar.activation(scr[:, :], scr[:, :], mybir.ActivationFunctionType.Sin)
            scalar.wait_ge(v1, 1)
            scalar.activation(cs[:, 0:512], red[:, 0:512], mybir.ActivationFunctionType.Sin,
                              scale=TWO_PI).then_inc(a1, 1)
            scalar.wait_ge(v1, 2)
            scalar.activation(cs[:, 512:WCS], red[:, 512:WCS], mybir.ActivationFunctionType.Sin,
                              scale=TWO_PI).then_inc(a1, 1)
            scalar.wait_ge(m1, 1)
            scalar.copy(outs[:, :], ps[:, :])
            scalar.dma_start(out=out[:, :], in_=outs[:, :]).then_inc(g1, 16)


        @block.gpsimd
        def _(gpsimd):
            gpsimd.wait_ge(g1, 16)
            # restore sem/DMA state for re-execution without a full barrier:
            # every other engine's last effect was already awaited on this chain
            lo = min(s.num for s in (d1, p1, v1, a1, m1, c1, g1))
            hi = max(s.num for s in (d1, p1, v1, a1, m1, c1, g1))
            gpsimd.dma_reset(range(lo, hi + 1))
            gpsimd.sem_clear(range(lo, hi + 1))

        # manual block exit: branch every engine to end_bb + per-engine drain,
        # but skip the all-engine event-semaphore barrier (costs ~2.5us; the
        # g1-gated sem_clear already guarantees a clean re-executable state)
        for engine, last_body in block.last_body.items():
            with nc.body(last_body, parent=nc.cur_bb, allow_existing_parent=True):
                engine.br(block.end_bb)
        nc.switch_bb(block.end_bb)
        gpsimd_type = nc.gpsimd.engine
        for eng_type, eng in nc.engines.items():
            if eng_type == gpsimd_type:
                continue
            dr = mybir.InstDrain(
                name=nc.get_next_instruction_name(), ins=[], outs=[],
                bass_is_fusable=False,
            )
            dr.engine = eng_type
            eng.add_instruction(dr)

    return nc


def _host_prep(q_re, q_im, p_re, p_im, x, psi):
    qf = q_re - p_im / f32(2.0)
    pf = f32(2.0) * q_im + p_re
    dq = f32((QMAX - QMIN) / QBINS)
    dp = f32((PMAX - PMIN) / PBINS)
    qb = np.floor((qf - f32(QMIN)) / dq)
    pb = np.floor((pf - f32(PMIN)) / dp)
    bins = (qb * PBINS + pb).astype(np.int32).reshape(-1)
    uniq, inv = np.unique(bins, return_inverse=True)
    qbi = qb.astype(np.int64).reshape(-1)
    pbi = pb.astype(np.int64).reshape(-1)
    qb_occ = np.unique(qbi)
    qb_rank = {v: i for i, v in enumerate(qb_occ)}
    qb_row = np.array([qb_rank[v] for v in qbi], dtype=np.int64)
    qc_occ = (qb_occ.astype(f32) + f32(0.5)) * dq + f32(QMIN)
    pc_all = (np.arange(PBINS, dtype=f32) + f32(0.5)) * dp + f32(PMIN)
    dx = np.diff(x)
    w = np.zeros_like(x)
    w[0] = dx[0] / 2
    w[-1] = dx[-1] / 2
    w[1:-1] = (dx[:-1] + dx[1:]) / 2
    wpsi = (w * psi).astype(f32)
    return bins, uniq, inv, qb_row, pbi, qc_occ, pc_all, wpsi


def _run_device(x, wpsi, qc_occ, pc_all, trace=False):
    M = x.shape[0]
    Qocc = qc_occ.shape[0]
    Q = max(8, int(np.ceil(Qocc / 8.0)) * 8)
    assert Q <= 128, "qb occupancy exceeds one PSUM partition tile"
    T = int(np.ceil(M / (N_CORES * 128.0)))
    Mp = N_CORES * T * 128
    CS = 2 * PBINS
    KK = 3 + 2 * T

    # grid is uniform: x[m] = x0 + m*h
    h = f32((float(x[-1]) - float(x[0])) / (M - 1))
    x0 = f32(x[0])

    wp = np.zeros(Mp, dtype=f32)
    wp[:M] = wpsi
    qc_pad = np.full(Q, 1000.0, dtype=f32)   # pad columns -> V = 0
    qc_pad[:Qocc] = qc_occ

    ws = wp.reshape(N_CORES, T, 128)

    key = (T, Q)
    if key not in _BUILD_CACHE:
        _BUILD_CACHE[key] = _build(T, Q)
    nc = _BUILD_CACHE[key]

    p_idx = np.arange(128, dtype=f32)
    r1 = (pc_all * h / f32(TWO_PI)).astype(f32)

    in_maps = []
    for c in range(N_CORES):
        # x[p, t] = a_t + h*p for this core's tiles
        a_t = (x0 + h * (np.arange(T, dtype=f32) + f32(c * T)) * f32(128.0)).astype(f32)
        wct = ws[c]                                   # [T, 128]
        # lhsT rows: ones, p, sgn x T (0.5 where wpsi<0), p^2, lnw x T (clamped)
        lhs = np.zeros((KK, 128), dtype=f32)
        lhs[0] = 1.0
        lhs[1] = p_idx
        lhs[2 + T] = p_idx * p_idx
        with np.errstate(divide="ignore"):
            lnw = np.log(np.abs(wct)).astype(f32)
        lnw = np.maximum(lnw, f32(-100.0))
        for t in range(T):
            lhs[2 + t] = f32(0.5) * (wct[t] < 0)
            lhs[3 + T + t] = lnw[t]
        rhs = np.zeros((KK, T * CS + T * Q), dtype=f32)
        for t in range(T):
            base = t * CS
            r0 = (pc_all * a_t[t] / f32(TWO_PI)).astype(f32)
            rhs[0, base:base + PBINS] = r0 + f32(0.25)   # cos args (quarter turn)
            rhs[0, base + PBINS:base + CS] = r0          # sin args
            rhs[1, base:base + CS] = np.concatenate([r1, r1])
            rhs[2 + t, base:base + CS] = 1.0             # sign half-turn for tile t
            qbase = T * CS + t * Q
            dqa = (qc_pad - a_t[t]).astype(f32)
            rhs[0, qbase:qbase + Q] = dqa * dqa
            rhs[1, qbase:qbase + Q] = f32(-2.0) * h * dqa
            rhs[2 + T, qbase:qbase + Q] = h * h
            rhs[3 + T + t, qbase:qbase + Q] = -1.0       # -ln|wpsi| for tile t
        in_maps.append({"coef": np.ascontiguousarray(np.concatenate([lhs, rhs], axis=1))})

    res = run_bass_kernel_spmd(nc, in_maps, core_ids=list(range(N_CORES)), trace=trace)
    F = np.zeros((Q, CS), dtype=np.float64)
    for c in range(N_CORES):
        F += res.results[c]["out"]
    F = F.astype(f32)
    return F[:Qocc, :PBINS], F[:Qocc, PBINS:], res


def kernel(factors_re, factors_im, q_re, q_im, p_re, p_im, x, psi):
    factors_re = np.asarray(factors_re, dtype=f32)
    factors_im = np.asarray(factors_im, dtype=f32)
    q_re = np.asarray(q_re, dtype=f32)
    q_im = np.asarray(q_im, dtype=f32)
    p_re = np.asarray(p_re, dtype=f32)
    p_im = np.asarray(p_im, dtype=f32)
    x = np.asarray(x, dtype=f32)
    psi = np.asarray(psi, dtype=f32)

    bins, uniq, inv, qb_row, pbi, qc_occ, pc_all, wpsi = _host_prep(
        q_re, q_im, p_re, p_im, x, psi
    )
    Fc, Fs, _ = _run_device(x, wpsi, qc_occ, pc_all)

    # ---- host tail: phase correction, gather, scatter-add, loss ----
    phi = (qc_occ[:, None] * pc_all[None, :]).astype(f32)
    cphi = np.cos(phi, dtype=f32)
    sphi = np.sin(phi, dtype=f32)
    G_re = f32(NORM) * (cphi * Fc + sphi * Fs)
    G_im = f32(NORM) * (sphi * Fc - cphi * Fs)
    gt_re = G_re[qb_row, pbi]
    gt_im = G_im[qb_row, pbi]

    e = np.exp((q_im * q_im).astype(f32), dtype=f32)
    ang = (p_re * q_im).astype(f32)
    pr = np.clip(np.nan_to_num(f32(NORM) * e * np.cos(ang, dtype=f32)), -100.0, 100.0).astype(f32)
    pi_ = np.clip(np.nan_to_num(f32(NORM) * e * np.sin(ang, dtype=f32)), -100.0, 100.0).astype(f32)
    vr = (pr * factors_re - pi_ * factors_im).astype(f32).reshape(-1)
    vi = (pr * factors_im + pi_ * factors_re).astype(f32).reshape(-1)

    N = vr.size
    B_re = np.zeros(N, dtype=f32)
    B_im = np.zeros(N, dtype=f32)
    np.add.at(B_re, inv, vr)
    np.add.at(B_im, inv, vi)
    dr = B_re - gt_re
    di = B_im - gt_im
    loss = np.sum(dr * dr + di * di, dtype=f32)
    return np.sqrt(loss, dtype=f32)

